# revision 3
# baseline (speedup 1.0000x reference)
"""Multi-head causal self-attention (B=4, T=2048, C=1024, 16 heads) on 8 trn2 cores.

Sharding: data-parallel over batch (4) x tensor-parallel over heads (2 groups of 8).
Core m handles batch m//2, head group m%2.

v2 design (vs baseline):
- No staging DMAs: DVE copies PSUM results straight into bf16 SBUF tiles
  (qT/kT/v/avT); only x/W loads, softmax-denominator broadcasts, the par1
  avT partition shift, and the output leave via DMA (the DMA device is a
  serialized resource in the cost model).
- Scores+AV matmuls run in bf16 (full PE rate at any moving width, so
  diagonal tiles cost exactly their width); projections stay f32r.
- Softmax denominators: ones column in v -> row 64 of the AV PSUM; DVE
  reciprocal + stride-0 DMA broadcast + DVE multiply (no ACT Ln/Exp, no
  PE ones-matmul).
- Phase interleave: QKV(s+1) / outproj(s-1) accumulation quanta are woven
  between attention chunk-pairs with a debt counter so the PE never idles
  on the exp (ACT) latency; ACT is ~78% of PE work total.
- Startup: x strip 0 and wq stream in per-c-chunk so the first matmul
  starts at ~2.5us instead of ~14.6us.
"""

import numpy as np

import concourse.bass as bass
import concourse.mybir as mybir
import concourse.tile as tile
from concourse.bass_utils import run_bass_kernel_spmd

F32 = mybir.dt.float32
F32R = mybir.dt.float32r
FP8 = mybir.dt.float8e4
PM = mybir.MatmulPerfMode
BF16 = mybir.dt.bfloat16
AF = mybir.ActivationFunctionType
MULT = mybir.AluOpType.mult
ADD = mybir.AluOpType.add

B, T, C = 4, 2048, 1024
HEADS, D = 16, 64
GROUPS = 2                  # head groups (tensor parallel)
HPC = HEADS // GROUPS       # heads per core = 8
GC = HPC * D                # group channel width = 512
NKC = T // 128              # Tk chunks = 16
NJ = T // 512               # Tq tiles = 4
CCH = C // 128              # contraction chunks = 8
NSTRIP = T // 512           # t strips = 4

_PROGRAM = None

# cost-model constants for the weave debt counter (ns)
PE_ROW = 0.4167             # ns per moving-dim row (full rate)
ACT_EL = 0.8333             # ns per free element on ACT
ACT_OH = 185.0              # fixed busy overhead per ACT instruction
QUANTUM = 4 * 512 * PE_ROW  # one dense quantum = 4 matmuls of N=512


def _patch_drain_chunking():
    """The axon walrus build rejects instructions with >~4 sem waits; Tile's
    kernel-tail drain waits on every live semaphore at once. Split it into a
    chain of drains with a bounded number of waits each."""
    from bass_rust import VectorClock, ScopedClock

    if getattr(tile.TileContext, "_drain_chunk_patched", False):
        return

    def _drain_and_barrier(self, tick_clock, wait_clock):
        gc_vec = list(tick_clock.global_clock)
        nz = [i for i, t in enumerate(gc_vec) if t > 0]
        CHUNK = 2
        for k in range(0, len(nz), CHUNK):
            keep = set(nz[k:k + CHUNK])
            partial = [gc_vec[i] if i in keep else 0 for i in range(len(gc_vec))]
            d = self.nc.sync.drain()
            wait_clock.add_sem_waits(d.ins, ScopedClock({None: VectorClock(partial)}))
        self.nc.all_engine_barrier()
        assert self.sems is not None
        popped = self.nc._tile_sem_poison_stack.pop()
        assert popped is self._sem_poison
        self.nc.clear_and_free_semaphores(list(self.sems.allocated().values()))
        self.nc.all_engine_barrier()

    tile.TileContext._drain_and_barrier = _drain_and_barrier
    tile.TileContext._drain_chunk_patched = True


def _split_excess_waits(nc, maxw=1, maxw_other=None):
    """Walrus rejects instructions carrying more than ~1 sem wait. Move excess
    waits onto same-engine NoOps inserted immediately before the instruction
    (engine streams execute in bb order, so semantics are preserved)."""
    from bass_rust import InstNoOp

    ctr = 0
    for f in nc.m.functions:
        for bb in f.blocks:
            new_insts = []
            for inst in bb.instructions:
                si = inst.sync_info
                waits = list(si.on_wait) if si and si.on_wait else []
                lim = maxw
                if maxw_other is not None and str(inst.engine) != 'EngineType.PE':
                    lim = maxw_other
                maxw_eff = lim
                if len(waits) > maxw_eff:
                    head, rest = waits[:-maxw_eff], waits[-maxw_eff:]
                    for k in range(0, len(head), maxw_eff):
                        ctr += 1
                        new_insts.append(InstNoOp(
                            name=f"waitnop_{ctr}",
                            engine=inst.engine,
                            sync_info=mybir.SyncInfo(
                                on_wait=head[k:k + maxw_eff], on_update=[]),
                        ))
                    inst.sync_info = mybir.SyncInfo(on_wait=rest, on_update=si.on_update)
                new_insts.append(inst)
            bb.instructions = new_insts
    return ctr


def _build_program():
    _patch_drain_chunking()
    nc = bass.Bass()

    xT_d = nc.declare_dram_parameter("xT", [C, T], BF16, isOutput=False)
    x8_d = nc.declare_dram_parameter("xT8", [C, T], FP8, isOutput=False)
    wq_d = nc.declare_dram_parameter("wqT", [C, GC], FP8, isOutput=False)
    wk_d = nc.declare_dram_parameter("wkT", [C, GC], FP8, isOutput=False)
    wv_d = nc.declare_dram_parameter("wvT", [C, GC], BF16, isOutput=False)
    wp_d = nc.declare_dram_parameter("wpT", [GC, C], BF16, isOutput=False)
    out_d = nc.declare_dram_parameter("outp", [T, C], F32, isOutput=True)

    from contextlib import ExitStack

    with tile.TileContext(nc) as tc, ExitStack() as stack:
        ep = stack.enter_context
        persist = ep(tc.tile_pool(name="persist", bufs=1))
        xs_pool = ep(tc.tile_pool(name="xs", bufs=2))
        pt_pool = ep(tc.tile_pool(name="pt", bufs=8))
        t8_pool = ep(tc.tile_pool(name="t8", bufs=3))
        avr_pool = ep(tc.tile_pool(name="avr", bufs=3))
        r_pool = ep(tc.tile_pool(name="rr", bufs=3))
        rb_pool = ep(tc.tile_pool(name="rb", bufs=3))
        avf_pool = ep(tc.tile_pool(name="avf", bufs=2))
        ob_pool = ep(tc.tile_pool(name="ob", bufs=2))
        dense_ps = ep(tc.tile_pool(name="dps", bufs=2, space="PSUM"))
        ps_s = ep(tc.tile_pool(name="pss", bufs=2, space="PSUM"))
        ps_av = ep(tc.tile_pool(name="psav", bufs=1, space="PSUM"))

        # per-strip q/k tiles, bf16 (scores operands stay bf16 for precision;
        # per-strip tiles because dep tracking is per-tile and one big tile
        # would serialize the weave)
        qTs = [persist.tile([128, HPC // 2, 512], BF16, name=f"qT_{s}")
               for s in range(NSTRIP)]
        kTs = [persist.tile([128, HPC // 2, 512], BF16, name=f"kT_{s}")
               for s in range(NSTRIP)]
        # v padded with a ones column per head: [t-chunk-part, chunk, head, 65]
        vs = [persist.tile([128, 4, HPC, D + 1], BF16, name=f"v_{s}")
              for s in range(NSTRIP)]
        for vt in vs:
            # memset first: ACT scale=0.0 on uninit garbage yields NaN*0=NaN on HW
            nc.gpsimd.memset(vt[:, :, :, D:D + 1], 0.0)
            nc.scalar.activation(vt[:, :, :, D:D + 1], vt[:, :, :, D:D + 1],
                                 AF.Copy, scale=0.0, bias=1.0)
        # avT split per (strip, head-pair): dep tracking is per-tile, so any
        # coarser layout makes outproj readers serialize behind the LATEST
        # writer of the shared tile (e.g. hp3's normalize), not their own
        avTs = [[persist.tile([128, 512], BF16, name=f"avT{s}_{h}")
                 for h in range(HPC // 2)] for s in range(NSTRIP)]

        # fp8 DoubleRow packing for q/k projections: input channel d=2*lane+i,
        # so a [256-row, N] DRAM block DMAs into [128, 2, N] in natural order
        wq = persist.tile([128, C // 256, 2, GC], FP8)
        wk = persist.tile([128, C // 256, 2, GC], FP8)
        wv = persist.tile([128, CCH, GC], BF16)
        wp = persist.tile([128, GC // 128, C], BF16)

        # ---- startup DMAs: x strip 0 + wq per c-chunk (interleaved), then wk/wv ----
        xs_tiles = {}
        x8_tiles = {}

        def load_strip8(s):
            t8x = xs_pool.tile([128, C // 256, 2, 512], FP8, tag="xs8", name=f"x8_{s}")
            x8_tiles[s] = t8x
            for c2 in range(C // 256):
                nc.sync.dma_start(
                    t8x[:, c2, :, :],
                    x8_d[256 * c2:256 * (c2 + 1), 512 * s:512 * (s + 1)]
                    .rearrange("(c p) t -> p c t", p=256))

        load_strip8(0)
        for c2 in range(C // 256):
            nc.sync.dma_start(
                wq[:, c2, :, :],
                wq_d[256 * c2:256 * (c2 + 1), :].rearrange("(c p) o -> p c o", p=256))
        for c2 in range(C // 256):
            nc.sync.dma_start(
                wk[:, c2, :, :],
                wk_d[256 * c2:256 * (c2 + 1), :].rearrange("(c p) o -> p c o", p=256))
        xs_tiles[0] = xs_pool.tile([128, CCH, 512], BF16, tag="xs", name="xs0")
        for c in range(CCH):
            nc.sync.dma_start(
                xs_tiles[0][:, c, :],
                xT_d[128 * c:128 * (c + 1), 0:512].rearrange("(c p) t -> p c t", p=128))
        def load_wv_wp():
            for c in range(CCH):
                nc.sync.dma_start(wv[:, c, :],
                                  wv_d[128 * c:128 * (c + 1), :]
                                  .rearrange("(c p) o -> p c o", p=128))
            for c in range(GC // 128):
                nc.sync.dma_start(wp[:, c, :],
                                  wp_d[128 * c:128 * (c + 1), :]
                                  .rearrange("(c p) o -> p c o", p=128))

        def load_strip(s):
            """Emit chunked DMA loads for x strip s (s>0)."""
            load_strip8(s)
            t = xs_pool.tile([128, CCH, 512], BF16, tag="xs", name=f"xs{s}")
            xs_tiles[s] = t
            for c in range(CCH):
                nc.sync.dma_start(
                    t[:, c, :],
                    xT_d[128 * c:128 * (c + 1), 512 * s:512 * (s + 1)]
                    .rearrange("(c p) t -> p c t", p=128))

        # ---- dense work generators (each yield = ~one QUANTUM of PE work) ----
        def gen_qk(s, o, w_sb, dsts):
            """q or k projection group: out channels [128o:128o+128] of strip s.
            Output is staged to fp8 and DMA'd into the DoubleRow-packed layout
            (d=2*lane+half interleave makes it a natural-order copy)."""
            x8 = x8_tiles[s]
            dst = dsts[s]
            pq = dense_ps.tile([128, 512], F32, tag="dp", name="pq")
            for c2 in range(C // 256):
                nc.tensor.matmul(pq[:, :], w_sb[:, c2, :, 128 * o:128 * (o + 1)],
                                 x8[:, c2, :, :], start=(c2 == 0),
                                 stop=(c2 == C // 256 - 1), perf_mode=PM.DoubleRow)
            if s == 0:
                nc.scalar.activation(dst[:, o, :], pq[:, :], AF.Copy)
            else:
                nc.vector.tensor_copy(dst[:, o, :], pq[:, :])
            yield 427.0

        def gen_v(s, tt):
            """v projection group: t chunk 4s+tt, all 8 heads."""
            xs = xs_tiles[s]
            pv = dense_ps.tile([128, 512], F32, tag="dp", name="pv")
            for c in range(4):
                nc.tensor.matmul(pv[:, :], xs[:, c, 128 * tt:128 * (tt + 1)],
                                 wv[:, c, :], start=(c == 0), stop=False)
            yield
            for c in range(4, CCH):
                nc.tensor.matmul(pv[:, :], xs[:, c, 128 * tt:128 * (tt + 1)],
                                 wv[:, c, :], start=False, stop=(c == CCH - 1))
            if s == 0:
                nc.scalar.activation(vs[s][:, tt, :, 0:D],
                                     pv[:, :].rearrange("p (h d) -> p h d", h=HPC), AF.Copy)
            else:
                nc.vector.tensor_copy(
                    vs[s][:, tt, :, 0:D],
                    pv[:, :].rearrange("p (h d) -> p h d", h=HPC))
            yield

        deferred_dmas = []

        def gen_po(tt, pool=None, cp="dve", defer_dma=False):
            """output projection for t chunk tt (both 512-col halves)."""
            pool = pool or dense_ps
            tail = pool is not dense_ps
            ob = ob_pool.tile([128, C], F32, tag="ob", name="ob")
            for o2 in range(2):
                po = pool.tile([128, 512], F32, tag="dp" if not tail else "s",
                               name="po")
                tl = tt % 4
                for c4 in range(GC // 128):
                    nc.tensor.matmul(po[:, :],
                                     avTs[tt // 4][c4][:, 128 * tl:128 * (tl + 1)],
                                     wp[:, c4, 512 * o2:512 * (o2 + 1)],
                                     start=(c4 == 0), stop=(c4 == GC // 128 - 1))
                if tail:
                    if o2 == 0:
                        nc.scalar.activation(ob[:, 512 * o2:512 * (o2 + 1)], po[:, :], AF.Copy)
                    else:
                        nc.vector.tensor_copy(ob[:, 512 * o2:512 * (o2 + 1)], po[:, :])
                    nc.sync.dma_start(
                        out_d[128 * tt:128 * (tt + 1), 512 * o2:512 * (o2 + 1)],
                        ob[:, 512 * o2:512 * (o2 + 1)])
                elif cp == "act":
                    nc.scalar.activation(ob[:, 512 * o2:512 * (o2 + 1)], po[:, :], AF.Copy)
                else:
                    nc.vector.tensor_copy(ob[:, 512 * o2:512 * (o2 + 1)], po[:, :])
                yield
            if not tail:
                if defer_dma:
                    deferred_dmas.append((tt, ob))
                else:
                    for o2 in range(2):
                        nc.sync.dma_start(
                            out_d[128 * tt:128 * (tt + 1), 512 * o2:512 * (o2 + 1)],
                            ob[:, 512 * o2:512 * (o2 + 1)])

        # ---- weave machinery ----
        queue = []          # pending dense generators (FIFO)
        debt = [0.0]

        def weave(extra_ns):
            debt[0] += extra_ns
            while debt[0] >= QUANTUM * 0.5 and queue:
                g = queue[0]
                try:
                    cost = next(g)
                    debt[0] -= cost if cost else QUANTUM
                except StopIteration:
                    queue.pop(0)

        def flush():
            while queue:
                g = queue.pop(0)
                for _ in g:
                    pass
            debt[0] = 0.0

        def enqueue_qkv(s):
            for o in range(HPC // 2):
                queue.append(gen_qk(s, o, wq, qTs))
            for o in range(HPC // 2):
                queue.append(gen_qk(s, o, wk, kTs))
            for tt in range(4):
                queue.append(gen_v(s, tt))

        # ---- attention ----
        def emit_att(j, hp):
            nkc = 4 * (j + 1)
            # both pars live in one 2-bank PSUM tile (cols [0:512] / [512:1024])
            av = ps_av.tile([65, 1024], F32, tag="av", name="av")

            def s_pair(i):
                roff = max(0, 128 * i - 512 * j)
                diag = 128 * i - 512 * j >= 0
                sps = ps_s.tile([128, 1024], F32, tag="s", name="sps")
                for par in range(2):
                    pb = 64 * par
                    nc.tensor.matmul(
                        sps[:, 512 * par + roff:512 * (par + 1)],
                        kTs[i // 4][pb:pb + 64, hp, 128 * (i % 4):128 * (i % 4 + 1)],
                        qTs[j][pb:pb + 64, hp, roff:512],
                        start=True, stop=True)
                ptile = pt_pool.tile([128, 1024], BF16, tag="pt", name="pt")
                # one exp for both pars: strided AP over the two [roff:512] ranges
                nc.scalar.activation(
                    ptile.rearrange("p (a b) -> p a b", a=2)[:, :, roff:512],
                    sps.rearrange("p (a b) -> p a b", a=2)[:, :, roff:512],
                    AF.Exp, scale=0.125)
                if diag:
                    for par in range(2):
                        # causal mask: zero p above the diagonal (Pool, SBUF-only)
                        nc.gpsimd.affine_select(
                            out=ptile[:, 512 * par + roff:512 * par + roff + 128],
                            in_=ptile[:, 512 * par + roff:512 * par + roff + 128],
                            compare_op=mybir.AluOpType.is_ge, fill=0.0, base=0,
                            pattern=[[1, 128]], channel_multiplier=-1,
                        )
                return (ptile, roff)

            def av_pair(i, pts):
                ptile, roff = pts
                for par in range(2):
                    nc.tensor.matmul(
                        av[:, 512 * par + roff:512 * (par + 1)],
                        vs[i // 4][:, i % 4, 2 * hp + par, :],
                        ptile[:, 512 * par + roff:512 * (par + 1)],
                        start=(i == 0), stop=(i == nkc - 1))

            def deficit(i):
                roff = max(0, 128 * i - 512 * j)
                n = 512 - roff
                act = ACT_OH + ACT_EL * 2 * n
                pe = 4 * n * PE_ROW
                return act - pe

            prev = s_pair(0)
            for i in range(1, nkc):
                cur = s_pair(i)
                weave(deficit(i - 1))
                av_pair(i - 1, prev)
                prev = cur
            weave(deficit(nkc - 1))
            av_pair(nkc - 1, prev)

            # normalize: denom at row 64 of av PSUM (ones column of v).
            # One copy/reciprocal/broadcast covers both pars; par1's multiply
            # goes first (it has the extra partition-shift DMA on its path).
            # Multiplies run on Pool (all-SBUF) so the DVE stream never gates
            # the dense-bank copies; the last strip stages via ACT to keep
            # DVE out of the tail chain entirely.
            last = (j == NJ - 1)
            if last:
                # tail path: DVE is idle here — skip staging, read av straight
                # from PSUM, per-par chains with par1 (the DMA-shifted one) first
                r = r_pool.tile([65, 1024], F32, tag="r", name="r")
                rb = rb_pool.tile([64, 1024], F32, tag="rb", name="rb")
                for par in (1, 0):
                    cs = slice(512 * par, 512 * (par + 1))
                    nc.vector.reciprocal(r[64:65, cs], av[64:65, cs])
                    nc.sync.dma_start(
                        rb[0:64, cs],
                        r[64:65, cs].unsqueeze(1).broadcast_to([1, 64, 512]))
                avf = avf_pool.tile([64, 512], BF16, tag="avf", name="avf")
                nc.vector.tensor_tensor(avf[:, :], av[0:64, 512:1024],
                                        rb[0:64, 512:1024], op=MULT)
                nc.sync.dma_start(avTs[j][hp][64:128, :], avf[:, :])
                nc.vector.tensor_tensor(
                    avTs[j][hp][0:64, :],
                    av[0:64, 0:512], rb[0:64, 0:512], op=MULT)
            else:
                asrc = avr_pool.tile([65, 1024], F32, tag="avr", name="avr")
                nc.vector.tensor_copy(asrc[:, :], av[:, :])
                r = r_pool.tile([65, 1024], F32, tag="r", name="r")
                nc.vector.reciprocal(r[64:65, :], asrc[64:65, :])
                rb = rb_pool.tile([64, 1024], F32, tag="rb", name="rb")
                nc.sync.dma_start(
                    rb[0:64, :], r[64:65, :].unsqueeze(1).broadcast_to([1, 64, 1024]))
                avf = avf_pool.tile([64, 512], BF16, tag="avf", name="avf")
                nc.gpsimd.tensor_tensor(avf[:, :], asrc[0:64, 512:1024],
                                        rb[0:64, 512:1024], op=MULT)
                nc.sync.dma_start(avTs[j][hp][64:128, :], avf[:, :])
                nc.gpsimd.tensor_tensor(
                    avTs[j][hp][0:64, :],
                    asrc[0:64, 0:512], rb[0:64, 0:512], op=MULT)

        # ---- main schedule ----
        # QKV(0): interleave pairs of groups so the first group doesn't
        # solo-wait on all 8 chunk DMAs
        # q/k groups first so their packing DMAs reach the device before the
        # wv/wp streams (ATT(0) waits on them); wv/wp issue just before v groups
        def run_pairs(gens):
            for a, b in zip(gens[0::2], gens[1::2]):
                for g in (a, b, a, b):
                    try:
                        next(g)
                    except StopIteration:
                        pass
        run_pairs([gen_qk(0, o, wq, qTs) for o in range(HPC // 2)]
                  + [gen_qk(0, o, wk, kTs) for o in range(HPC // 2)])
        load_wv_wp()
        run_pairs([gen_v(0, tt) for tt in range(4)])
        for j in range(NJ):
            if j + 1 < NSTRIP:
                load_strip(j + 1)      # x strip j+1 streams during ATT(j)
                enqueue_qkv(j + 1)     # woven into ATT(j)
            reserve = []
            if j == NJ - 1:
                for tt in range(0, 4 * j - 2):
                    queue.append(gen_po(tt))   # PO(0..2) woven into ATT(3)
                reserve = [gen_po(tt, cp="act") for tt in range(4 * j - 2, 4 * j)]
            for hp in range(HPC // 2):
                emit_att(j, hp)
            for g in reserve:          # fills the last hp's normalize latency
                for _ in g:
                    pass
            for tt, ob in deferred_dmas:   # now queue behind the hp3 rb/avf DMAs
                for o2 in range(2):
                    nc.sync.dma_start(
                        out_d[128 * tt:128 * (tt + 1), 512 * o2:512 * (o2 + 1)],
                        ob[:, 512 * o2:512 * (o2 + 1)])
            deferred_dmas.clear()
            if j + 1 < NSTRIP:
                flush()                # QKV(j+1) must be fully emitted before ATT(j+1)
        flush()
        for tt in range(4 * (NJ - 1), 4 * NJ):
            g = gen_po(tt, pool=ps_s)
            for _ in g:
                pass
    _split_excess_waits(nc)
    return nc


def _get_program():
    global _PROGRAM
    if _PROGRAM is None:
        _PROGRAM = _build_program()
    return _PROGRAM


def _make_in_maps(x, Wk, Wq, Wv, Wp):
    import ml_dtypes
    x = np.asarray(x, dtype=np.float32)
    Wk = np.asarray(Wk, dtype=np.float32)
    Wq = np.asarray(Wq, dtype=np.float32)
    Wv = np.asarray(Wv, dtype=np.float32)
    Wp = np.asarray(Wp, dtype=np.float32)
    in_maps = []
    for core in range(8):
        b, g = core // GROUPS, core % GROUPS
        rows = slice(GC * g, GC * (g + 1))
        in_maps.append({
            "xT": np.ascontiguousarray(x[b].T).astype(ml_dtypes.bfloat16),
            "xT8": np.ascontiguousarray(x[b].T).astype(ml_dtypes.float8_e4m3),
            "wqT": np.ascontiguousarray(Wq[rows, :].T).astype(ml_dtypes.float8_e4m3),
            "wkT": np.ascontiguousarray(Wk[rows, :].T).astype(ml_dtypes.float8_e4m3),
            "wvT": np.ascontiguousarray(Wv[rows, :].T).astype(ml_dtypes.bfloat16),
            "wpT": np.ascontiguousarray(Wp[:, rows].T).astype(ml_dtypes.bfloat16),
        })
    return in_maps


def run(x, Wk, Wq, Wv, Wp, bp, trace=False, **spmd_kwargs):
    nc = _get_program()
    in_maps = _make_in_maps(x, Wk, Wq, Wv, Wp)
    res = run_bass_kernel_spmd(nc, in_maps, list(range(8)), trace=trace, **spmd_kwargs)
    bp = np.asarray(bp, dtype=np.float32)
    out = np.empty((B, T, C), dtype=np.float32)
    for b in range(B):
        out[b] = res.results[GROUPS * b]["outp"] + res.results[GROUPS * b + 1]["outp"] + bp
    return out, res


def kernel(x, Wk, Wq, Wv, Wp, bp):
    out, _ = run(x, Wk, Wq, Wv, Wp, bp)
    return out
